# revision 1
# baseline (speedup 1.0000x reference)
"""CRF path-score kernel for Trainium2 (8 NeuronCores, Bass/Tile).

score = -( sum_i logits[i, tags[i]] + sum_{i>0} transitions[tags[i-1], tags[i]] )

Strategy (hardcoded for S=2_000_000, T=50 tags, 8 cores):
  - Shard the sequence across 8 cores (250k rows each, padded to 128*1960).
  - Layout per core: partition p holds 1960 consecutive rows ("chunks" g),
    logits as [128, 1960*50] bf16 (host-cast), tags as [128, 1961] bf16
    (col 1960 = next partition's first tag; out-of-range pad tag 63 -> all-zero
    one-hot).
  - Device: DVE builds the one-hot matrix E (is_equal against an iota tile);
    per chunk g ONE matmul with stationary E_g [128,50] and moving
    [L_g | E_{g+1}] (custom strided AP, N=100) accumulates into PSUM [50,100]:
      cols  0:50  += E_g^T L_g      (M: per-tag logit sums; trace(M) = emission)
      cols 50:100 += E_g^T E_{g+1}  (C: transition pair counts, exact ints)
  - Host: score = -(trace(M) + sum(C*T) + 7 core-boundary pairs).
"""

import dataclasses
import sys
import types

if "/opt/trn_rl_repo" not in sys.path:
    sys.path.insert(0, "/opt/trn_rl_repo")

import ml_dtypes
import numpy as np

S = 2_000_000
T = 50
N_CORES = 8
R = S // N_CORES          # 250_000 rows per core
G = 1960                  # chunks per partition (128*1960 = 250_880 >= R)
GB = 140                  # chunks per block
NB = G // GB              # 14 blocks
PAD_TAG = 63              # out of range -> zero one-hot
EOFF = GB * 50            # start of E region in the combined tile (7000)
CW = EOFF + (GB + 1) * 50 # combined tile width (14050)

_CACHE = {}


def _build_nc():
    import concourse.tile as tile
    import concourse.mybir as mybir
    from concourse import bacc
    from contextlib import ExitStack

    nc = bacc.Bacc("TRN2", target_bir_lowering=False, debug=False, num_devices=1)
    bf16 = mybir.dt.bfloat16
    f32 = mybir.dt.float32

    logits_d = nc.dram_tensor("logits", [128, G * 50], bf16, kind="ExternalInput").ap()
    tags_d = nc.dram_tensor("tags", [128, G + 1], bf16, kind="ExternalInput").ap()
    iota_d = nc.dram_tensor("iota", [128, 50], bf16, kind="ExternalInput").ap()
    cm_d = nc.dram_tensor("cm", [50, 100], f32, kind="ExternalOutput").ap()

    with tile.TileContext(nc) as tc, ExitStack() as ctx:
        cpool = ctx.enter_context(tc.tile_pool(name="comb", bufs=3))
        const = ctx.enter_context(tc.tile_pool(name="const", bufs=1))
        ppool = ctx.enter_context(tc.tile_pool(name="psum", bufs=1, space="PSUM"))
        opool = ctx.enter_context(tc.tile_pool(name="out", bufs=1))

        tags_t = const.tile([128, G + 1], bf16)
        nc.sync.dma_start(tags_t[:], tags_d[:])
        iota_t = const.tile([128, 50], bf16)
        nc.sync.dma_start(iota_t[:], iota_d[:])

        psum = ppool.tile([50, 100], f32)

        for b in range(NB):
            comb = cpool.tile([128, CW], bf16)
            # logits block: per-partition contiguous 14 KB -> efficient DMA
            nc.sync.dma_start(comb[:, 0:EOFF], logits_d[:, b * EOFF:(b + 1) * EOFF])
            # build one-hot E for GB+1 chunks in one DVE op
            E3v = comb[:, EOFF:CW].rearrange("p (g c) -> p g c", g=GB + 1, c=50)
            t3 = tags_t[:, b * GB:b * GB + GB + 1, None].to_broadcast([128, GB + 1, 50])
            i3 = iota_t[:, None, :].to_broadcast([128, GB + 1, 50])
            nc.vector.tensor_tensor(E3v, t3, i3, mybir.AluOpType.is_equal)

            for g in range(GB):
                lhsT = comb[:, EOFF + g * 50:EOFF + (g + 1) * 50]
                base = comb[:, g * 50:(g + 1) * 50]
                # moving operand [L_g | E_{g+1}]: two 50-wide groups, stride EOFF+50
                rhs = dataclasses.replace(base, ap=[base.ap[0], [EOFF + 50, 2], [1, 50]])
                nc.tensor.matmul(
                    psum[:, :], lhsT, rhs,
                    start=(b == 0 and g == 0),
                    stop=(b == NB - 1 and g == GB - 1),
                )

        out_t = opool.tile([50, 100], f32)
        nc.vector.tensor_copy(out_t[:], psum[:, :])
        nc.sync.dma_start(cm_d[:], out_t[:])

    nc.compile()
    return nc


def _get_nc():
    if "nc" not in _CACHE:
        _CACHE["nc"] = _build_nc()
    return _CACHE["nc"]


def _prepare_in_maps(logits, tags_i):
    iota_bf = np.tile(np.arange(50, dtype=np.float32), (128, 1)).astype(ml_dtypes.bfloat16)
    in_maps = []
    for k in range(N_CORES):
        tk = tags_i[k * R:(k + 1) * R]
        tk_pad = np.full(128 * G, PAD_TAG, np.int64)
        tk_pad[:R] = tk
        tg2 = tk_pad.reshape(128, G)
        tags_ext = np.full((128, G + 1), PAD_TAG, np.int64)
        tags_ext[:, :G] = tg2
        tags_ext[:127, G] = tg2[1:, 0]  # halo: next partition's first tag
        tags_bf = tags_ext.astype(np.float32).astype(ml_dtypes.bfloat16)

        lk = logits[k * R:(k + 1) * R]
        lk_pad = np.zeros((128 * G, 50), np.float32)
        lk_pad[:R] = lk
        logits_bf = lk_pad.astype(ml_dtypes.bfloat16).reshape(128, G * 50)

        in_maps.append({"logits": logits_bf, "tags": tags_bf, "iota": iota_bf})
    return in_maps


def run_device(logits, tags_i, trace=False, **kw):
    """Compile (cached) + run on 8 cores. Returns BassKernelResults."""
    from concourse.bass_utils import run_bass_kernel_spmd

    nc = _get_nc()
    in_maps = _prepare_in_maps(logits, tags_i)
    return run_bass_kernel_spmd(
        nc, in_maps, core_ids=list(range(N_CORES)), trace=trace, **kw
    )


def combine(results, tags_i, transitions):
    M_sum = np.zeros((50, 50), np.float64)
    C_sum = np.zeros((50, 50), np.float64)
    for k in range(N_CORES):
        cm = results[k]["cm"].astype(np.float64)
        M_sum += cm[:, :50]
        C_sum += cm[:, 50:]
    trans64 = np.asarray(transitions, dtype=np.float64)
    emit = np.trace(M_sum)
    trans = float((C_sum * trans64).sum())
    for k in range(1, N_CORES):  # core-boundary pairs
        trans += float(trans64[tags_i[k * R - 1], tags_i[k * R]])
    return np.asarray(-(emit + trans), dtype=np.float32)


def kernel(logits, tags, transitions):
    logits = np.asarray(logits, dtype=np.float32)
    tags_i = np.asarray(tags).astype(np.int64)
    res = run_device(logits, tags_i, trace=False)
    return combine(res.results, tags_i, transitions)


# revision 3
# speedup vs baseline: 1.0538x; 1.0538x over previous
"""CRF path-score kernel for Trainium2 (8 NeuronCores, Bass/Tile).

score = -( sum_i logits[i, tags[i]] + sum_{i>0} transitions[tags[i-1], tags[i]] )

Strategy (hardcoded for S=2_000_000, T=50 tags, 8 cores):
  - Shard the sequence across 8 cores (250k rows each, padded to 128*1960).
  - Layout per core: partition p holds 1960 consecutive rows ("chunks" g),
    logits as [128, 1960*50] bf16 (host-cast), tags as [128, 1961] bf16
    (col 1960 = next partition's first tag; out-of-range pad tag 63 -> all-zero
    one-hot).
  - Device: DVE builds the one-hot matrix E (is_equal against an iota tile);
    per chunk g ONE matmul with stationary E_g [128,50] and moving
    [L_g | E_{g+1}] (custom strided AP, N=100) accumulates into PSUM [50,100]:
      cols  0:50  += E_g^T L_g      (M: per-tag logit sums; trace(M) = emission)
      cols 50:100 += E_g^T E_{g+1}  (C: transition pair counts, exact ints)
  - Host: score = -(trace(M) + sum(C*T) + 7 core-boundary pairs).
"""

import dataclasses
import sys
import types

if "/opt/trn_rl_repo" not in sys.path:
    sys.path.insert(0, "/opt/trn_rl_repo")

import ml_dtypes
import numpy as np

S = 2_000_000
T = 50
N_CORES = 8
R = S // N_CORES          # 250_000 rows per core
G = 1960                  # chunks per partition (128*1960 = 250_880 >= R)
GB = 140                  # chunks per block
NB = G // GB              # 14 blocks
PAD_TAG = 63              # out of range -> zero one-hot
EOFF = GB * 50            # start of E region in the combined tile (7000)
CW = EOFF + (GB + 1) * 50 # combined tile width (14050)

_CACHE = {}


def _build_nc():
    import concourse.tile as tile
    import concourse.mybir as mybir
    from concourse import bacc
    from contextlib import ExitStack

    nc = bacc.Bacc("TRN2", target_bir_lowering=False, debug=False, num_devices=1)
    bf16 = mybir.dt.bfloat16
    f32 = mybir.dt.float32

    logits_d = nc.dram_tensor("logits", [128, G * 50], bf16, kind="ExternalInput").ap()
    tags_d = nc.dram_tensor("tags", [128, G + 1], bf16, kind="ExternalInput").ap()
    iota_d = nc.dram_tensor("iota", [128, (GB + 1) * 50], bf16, kind="ExternalInput").ap()
    cm_d = nc.dram_tensor("cm", [50, 100], f32, kind="ExternalOutput").ap()

    with tile.TileContext(nc) as tc, ExitStack() as ctx:
        cpool = ctx.enter_context(tc.tile_pool(name="comb", bufs=3))
        rpool = ctx.enter_context(tc.tile_pool(name="rep", bufs=3))
        const = ctx.enter_context(tc.tile_pool(name="const", bufs=1))
        ppool = ctx.enter_context(tc.tile_pool(name="psum", bufs=1, space="PSUM"))
        opool = ctx.enter_context(tc.tile_pool(name="out", bufs=1))

        tags_t = const.tile([128, G + 1], bf16)
        nc.sync.dma_start(tags_t[:], tags_d[:])
        iota_t = const.tile([128, (GB + 1) * 50], bf16)  # tiled iota pattern
        nc.sync.dma_start(iota_t[:], iota_d[:])

        psum = ppool.tile([50, 100], f32)

        for b in range(NB):
            comb = cpool.tile([128, CW], bf16)
            # logits block: per-partition contiguous 14 KB -> efficient DMA
            nc.sync.dma_start(comb[:, 0:EOFF], logits_d[:, b * EOFF:(b + 1) * EOFF])
            # ACT materializes the repeated tags (idle engine), so the DVE
            # compare below sees two dense step-1 bf16 operands -> 2x mode
            rep = rpool.tile([128, (GB + 1) * 50], bf16)
            rep3 = rep[:, :].rearrange("p (g c) -> p g c", g=GB + 1, c=50)
            t3 = tags_t[:, b * GB:b * GB + GB + 1, None].to_broadcast([128, GB + 1, 50])
            nc.scalar.copy(rep3, t3)
            nc.vector.tensor_tensor(
                comb[:, EOFF:CW], rep[:, :], iota_t[:, :], mybir.AluOpType.is_equal
            )

            for g in range(GB):
                lhsT = comb[:, EOFF + g * 50:EOFF + (g + 1) * 50]
                base = comb[:, g * 50:(g + 1) * 50]
                # moving operand [L_g | E_{g+1}]: two 50-wide groups, stride EOFF+50
                rhs = dataclasses.replace(base, ap=[base.ap[0], [EOFF + 50, 2], [1, 50]])
                nc.tensor.matmul(
                    psum[:, :], lhsT, rhs,
                    start=(b == 0 and g == 0),
                    stop=(b == NB - 1 and g == GB - 1),
                )

        out_t = opool.tile([50, 100], f32)
        nc.vector.tensor_copy(out_t[:], psum[:, :])
        nc.sync.dma_start(cm_d[:], out_t[:])

    nc.compile()
    return nc


def _get_nc():
    if "nc" not in _CACHE:
        _CACHE["nc"] = _build_nc()
    return _CACHE["nc"]


def _prepare_in_maps(logits, tags_i):
    iota_bf = np.tile(np.arange(50, dtype=np.float32), (128, GB + 1)).astype(ml_dtypes.bfloat16)
    in_maps = []
    for k in range(N_CORES):
        tk = tags_i[k * R:(k + 1) * R]
        tk_pad = np.full(128 * G, PAD_TAG, np.int64)
        tk_pad[:R] = tk
        tg2 = tk_pad.reshape(128, G)
        tags_ext = np.full((128, G + 1), PAD_TAG, np.int64)
        tags_ext[:, :G] = tg2
        tags_ext[:127, G] = tg2[1:, 0]  # halo: next partition's first tag
        tags_bf = tags_ext.astype(np.float32).astype(ml_dtypes.bfloat16)

        lk = logits[k * R:(k + 1) * R]
        lk_pad = np.zeros((128 * G, 50), np.float32)
        lk_pad[:R] = lk
        logits_bf = lk_pad.astype(ml_dtypes.bfloat16).reshape(128, G * 50)

        in_maps.append({"logits": logits_bf, "tags": tags_bf, "iota": iota_bf})
    return in_maps


def run_device(logits, tags_i, trace=False, **kw):
    """Compile (cached) + run on 8 cores. Returns BassKernelResults."""
    from concourse.bass_utils import run_bass_kernel_spmd

    nc = _get_nc()
    in_maps = _prepare_in_maps(logits, tags_i)
    return run_bass_kernel_spmd(
        nc, in_maps, core_ids=list(range(N_CORES)), trace=trace, **kw
    )


def combine(results, tags_i, transitions):
    M_sum = np.zeros((50, 50), np.float64)
    C_sum = np.zeros((50, 50), np.float64)
    for k in range(N_CORES):
        cm = results[k]["cm"].astype(np.float64)
        M_sum += cm[:, :50]
        C_sum += cm[:, 50:]
    trans64 = np.asarray(transitions, dtype=np.float64)
    emit = np.trace(M_sum)
    trans = float((C_sum * trans64).sum())
    for k in range(1, N_CORES):  # core-boundary pairs
        trans += float(trans64[tags_i[k * R - 1], tags_i[k * R]])
    return np.asarray(-(emit + trans), dtype=np.float32)


def kernel(logits, tags, transitions):
    logits = np.asarray(logits, dtype=np.float32)
    tags_i = np.asarray(tags).astype(np.int64)
    res = run_device(logits, tags_i, trace=False)
    return combine(res.results, tags_i, transitions)
